# revision 6
# baseline (speedup 1.0000x reference)
"""CGNN layer kernel for Trainium2 (8 NeuronCores, SPMD) — v2.

Sharding: core c owns batch b = c//2 and receiver-node half i0 = (c%2)*128.
Each core computes its (128, 128) output shard from full-j message passing.

Math per core (receivers i, live senders j compacted to L <= npad):
  z[h,(i,j)] = W1d adj[i,j] + ACb[h,i] + base[h,j]
  S[h,i]     = sum_j silu(z)            (padded j add silu(ACb); folded into d)
  aggr       = W2 S - d,  d = W2*(npad-L)*silu(ACb) - L*b2     (host)
  u          = silu(W3b aggr + e),  e = W3a x_i + b3           (host)
  y          = W4 u + xb,          xb = x_i + b4               (host)
  out[h,i]   = LN_h(y) * gamma + beta   (host un-transposes to [i,h])

Device structure:
  - ALL constants host-precomputed (no setup chain, no transposes on device).
  - Warmup matmuls ramp the PE p-state during the input-DMA phase; a tiny
    silu preloads the ACT table.
  - Main loop: 2-chunk groups; per chunk two accumulating matmuls
    (K=128 base via broadcast rhs, K=32+nr adj+ACb via onehot rows), one
    1024-col silu per group (ACT, spans 2 PSUM banks), one bf16 4x-mode
    tensor_reduce per group (DVE).
  - Epilogue split into column groups overlapped with the loop; LayerNorm
    stats via GpSimd partition-reduce; rstd via bitcast fast-rsqrt on DVE
    (avoids the Sqrt ACT-table reload).
"""

import numpy as np
import ml_dtypes
ml_bf16 = ml_dtypes.bfloat16
from contextlib import ExitStack

import concourse.bass as bass
import concourse.bass_isa as bass_isa
import concourse.bacc as bacc
import concourse.mybir as mybir
import concourse.tile as tile
from concourse.bass_utils import run_bass_kernel_spmd

B, N, H, R = 4, 256, 128, 32
NI = 128          # receivers per core
FP = mybir.dt.float32
BF = mybir.dt.bfloat16
I32 = mybir.dt.int32
ALU = mybir.AluOpType
ACTF = mybir.ActivationFunctionType
AXL = mybir.AxisListType

_cache = {}

NWARM = 6         # PE p-state warmup matmuls
NEWT = 2          # Newton iterations for fast-rsqrt (even => positive rstd)
EPI_G = 2         # epilogue receiver-column groups
MAGIC = 0x5F3759DF


def _build_program(npad, nr, nc_chunks):
    KB = 32 + nr
    W = nr * npad                        # rhs cols per chunk (<= 512)
    swidth = nc_chunks * nr              # S cols incl phantom slots
    ngrp = (nc_chunks + 1) // 2          # 2-chunk groups (last may be 1)
    G2 = NI // EPI_G                     # receivers per epilogue group

    nc = bacc.Bacc()

    # ---- DRAM parameters ----
    # bb1: first-needed bf16 consts: w1bT | xTm
    bb1 = nc.declare_dram_parameter("bb1", [H, H + npad], BF, isOutput=False)
    # bb2: epilogue bf16 consts: ident | w2T | w3bT | w4T | negd | e | xbT
    bb2 = nc.declare_dram_parameter("bb2", [H, 7 * H], BF, isOutput=False)
    # cb: fp32 per-partition scalars: gamma_eff | beta
    cb = nc.declare_dram_parameter("cb", [H, 2], FP, isOutput=False)
    # slabs: per-chunk [KB, 512 rhs | 128 lhsT] packed k-major
    CW = W + H
    slab_chunks = [min(2, nc_chunks)]
    while sum(slab_chunks) < nc_chunks:
        slab_chunks.append(min(4, nc_chunks - sum(slab_chunks)))
    slabs_par = []
    for s, cnt in enumerate(slab_chunks):
        slabs_par.append(nc.declare_dram_parameter(
            f"slab{s}", [KB, cnt * CW], BF, isOutput=False))
    out = nc.declare_dram_parameter("out", [H, NI], FP, isOutput=True)

    with ExitStack() as ctx:
        tc = ctx.enter_context(tile.TileContext(nc))
        const = ctx.enter_context(tc.tile_pool(name="const", bufs=1))
        work = ctx.enter_context(tc.tile_pool(name="work", bufs=2))
        sctp = ctx.enter_context(tc.tile_pool(name="sctp", bufs=3))
        pz = ctx.enter_context(tc.tile_pool(name="pz", bufs=3, space="PSUM"))
        pep = ctx.enter_context(tc.tile_pool(name="pep", bufs=2, space="PSUM"))

        # ---- const DMAs ----
        bbt1 = const.tile([H, H + npad], BF, tag="bbt1", name="bbt1")
        nc.sync.dma_start(out=bbt1, in_=bb1[:])
        w1bT = bbt1[:, 0:H]
        xTm = bbt1[:, H:H + npad]

        bbt2 = const.tile([H, 7 * H], BF, tag="bbt2", name="bbt2")
        nc.scalar.dma_start(out=bbt2, in_=bb2[:])
        ident = bbt2[:, 0:H]
        w2T = bbt2[:, H:2 * H]
        w3bT = bbt2[:, 2 * H:3 * H]
        w4T = bbt2[:, 3 * H:4 * H]
        negd = bbt2[:, 4 * H:5 * H]
        e_sb = bbt2[:, 5 * H:6 * H]
        xbT = bbt2[:, 6 * H:7 * H]

        cbt = const.tile([H, 2], FP, tag="cbt", name="cbt")
        nc.sync.dma_start(out=cbt, in_=cb[:])
        gam_col = cbt[:, 0:1]
        bet_col = cbt[:, 1:2]

        # slab DMAs: alternate sync / scalar / gpsimd queues, slab0 first
        slab_tiles = []
        engs = [nc.sync, nc.scalar, nc.gpsimd]
        eng_order = [0, 1, 2, 0, 2, 0, 2, 0, 1, 0, 2]
        c0 = 0
        for s, cnt in enumerate(slab_chunks):
            st = const.tile([KB, cnt, CW], BF, tag=f"slab{s}",
                            name=f"slab{s}")
            src = slabs_par[s][:].rearrange("k (c w) -> k c w", w=CW)
            engs[eng_order[s % len(eng_order)]].dma_start(out=st, in_=src)
            for c in range(cnt):
                slab_tiles.append((st, c))
            c0 += cnt

        # ---- warmup: ACT table preload + PE p-state ramp ----
        wt = const.tile([H, 512], BF, tag="wt", name="wt")
        nc.vector.memset(wt, 0.125)
        ws = const.tile([H, 1], BF, tag="ws", name="ws")
        nc.scalar.activation(ws, wt[:, 0:1], ACTF.Silu)
        for k in range(NWARM):
            wp = pep.tile([H, 512], FP, tag="pe2", name=f"wp{k}")
            nc.tensor.matmul(wp, lhsT=wt[:, 0:H], rhs=wt[:, :],
                             start=True, stop=True)

        # ---- main loop ----
        xTm_bc = bass.AP(tensor=xTm.tensor, offset=xTm.offset,
                         ap=[list(xTm.ap[0]), [0, nr]] +
                            [list(d) for d in xTm.ap[1:]])
        S = const.tile([H, swidth], BF, tag="S", name="S")

        for g in range(ngrp):
            cA = 2 * g
            nchunk = min(2, nc_chunks - cA)
            pzg = pz.tile([H, 2, 512], FP, tag="pz", name=f"pz{g}")
            for t in range(nchunk):
                c = cA + t
                st, ci = slab_tiles[c]
                rhs_c = st[:, ci, 0:W]
                lhsT_c = st[:, ci, W:W + H]
                dst = pzg[:, t, 0:W]
                nc.tensor.matmul(dst, lhsT=w1bT, rhs=xTm_bc,
                                 start=True, stop=False)
                nc.tensor.matmul(dst, lhsT=lhsT_c, rhs=rhs_c,
                                 start=False, stop=True)
            sct = sctp.tile([H, 2, W], BF, tag="sct", name=f"sct{g}")
            nc.scalar.activation(
                sct[:, 0:nchunk, :].rearrange("p a b -> p (a b)"),
                pzg[:, 0:nchunk, 0:W].rearrange("p a b -> p (a b)"),
                ACTF.Silu)
            with nc.allow_low_precision("bf16 S; j-sums small"):
                nc.vector.tensor_reduce(
                    out=S[:, cA * nr:(cA + nchunk) * nr],
                    in_=sct[:, 0:nchunk, :].rearrange(
                        "p a (e j) -> p (a e) j", j=npad),
                    axis=AXL.X, op=ALU.add)

        # ---- epilogue (per receiver-column group) ----
        outt = const.tile([H, NI], FP, tag="outt", name="outt")

        for eg in range(EPI_G):
            sl = slice(eg * G2, (eg + 1) * G2)
            pa = pep.tile([H, 512], FP, tag="pe2", name=f"pa{eg}")
            nc.tensor.matmul(pa[:, 0:G2], lhsT=w2T, rhs=S[:, sl],
                             start=True, stop=False)
            nc.tensor.matmul(pa[:, 0:G2], lhsT=ident, rhs=negd[:, sl],
                             start=False, stop=True)
            aggr = work.tile([H, G2], BF, tag="aggr", name=f"aggr{eg}")
            nc.scalar.activation(aggr, pa[:, 0:G2], ACTF.Copy)

            pu = pep.tile([H, 512], FP, tag="pe2", name=f"pu{eg}")
            nc.tensor.matmul(pu[:, 0:G2], lhsT=w3bT, rhs=aggr,
                             start=True, stop=False)
            nc.tensor.matmul(pu[:, 0:G2], lhsT=ident, rhs=e_sb[:, sl],
                             start=False, stop=True)
            u_bf = work.tile([H, G2], BF, tag="u_bf", name=f"u{eg}")
            nc.scalar.activation(u_bf, pu[:, 0:G2], ACTF.Silu)

            py = pep.tile([H, 512], FP, tag="pe2", name=f"py{eg}")
            nc.tensor.matmul(py[:, 0:G2], lhsT=w4T, rhs=u_bf,
                             start=True, stop=False)
            nc.tensor.matmul(py[:, 0:G2], lhsT=ident, rhs=xbT[:, sl],
                             start=False, stop=True)
            y_sb = work.tile([H, G2], FP, tag="y_sb", name=f"y{eg}")
            nc.scalar.activation(y_sb, py[:, 0:G2], ACTF.Copy)
            ysq = work.tile([H, G2], FP, tag="ysq", name=f"ysq{eg}")
            nc.vector.scalar_tensor_tensor(
                out=ysq, in0=py[:, 0:G2], scalar=0.0, in1=y_sb,
                op0=ALU.add, op1=ALU.mult)
            # partition all-reduce: every partition ends up with the h-sum
            musum = work.tile([H, G2], FP, tag="musum", name=f"musum{eg}")
            nc.gpsimd.partition_all_reduce(musum, y_sb, channels=H,
                                           reduce_op=bass_isa.ReduceOp.add)
            qsum = work.tile([H, G2], FP, tag="qsum", name=f"qsum{eg}")
            nc.gpsimd.partition_all_reduce(qsum, ysq, channels=H,
                                           reduce_op=bass_isa.ReduceOp.add)

            # v128 = 128*var = qsum - musum^2/128
            m2 = work.tile([H, G2], FP, tag="m2", name=f"m2{eg}")
            nc.vector.scalar_tensor_tensor(
                out=m2, in0=musum, scalar=0.0, in1=musum,
                op0=ALU.add, op1=ALU.mult)
            v128 = work.tile([H, G2], FP, tag="v128", name=f"v128{eg}")
            nc.vector.scalar_tensor_tensor(
                out=v128, in0=m2, scalar=-1.0 / H, in1=qsum,
                op0=ALU.mult, op1=ALU.add)
            # fast inverse sqrt: bitcast magic + NEWT Newton steps
            ri = work.tile([H, G2], I32, tag="ri", name=f"ri{eg}")
            nc.vector.tensor_scalar(ri, v128.bitcast(I32), 1, None,
                                    ALU.logical_shift_right)
            r0i = work.tile([H, G2], I32, tag="r0i", name=f"r0i{eg}")
            nc.vector.tensor_scalar(r0i, ri, MAGIC, -1,
                                    ALU.subtract, ALU.mult)
            r_prev = r0i.bitcast(FP)
            for it in range(NEWT):
                rr = work.tile([H, G2], FP, tag=f"rr{it}", name=f"rr{it}_{eg}")
                nc.vector.scalar_tensor_tensor(
                    out=rr, in0=r_prev, scalar=0.0, in1=r_prev,
                    op0=ALU.add, op1=ALU.mult)
                bb_ = work.tile([H, G2], FP, tag=f"bb{it}",
                                name=f"bb{it}_{eg}")
                nc.vector.scalar_tensor_tensor(
                    out=bb_, in0=rr, scalar=0.5, in1=v128,
                    op0=ALU.mult, op1=ALU.mult)
                rn = work.tile([H, G2], FP, tag=f"rn{it}",
                               name=f"rn{it}_{eg}")
                nc.vector.scalar_tensor_tensor(
                    out=rn, in0=bb_, scalar=1.5, in1=r_prev,
                    op0=ALU.subtract, op1=ALU.mult)
                r_prev = rn  # negated each iteration; NEWT even => positive

            # normalize: out = (H*y - musum) * rstd128 * gamma_eff + beta
            n1 = work.tile([H, G2], FP, tag="n1", name=f"n1{eg}")
            nc.vector.scalar_tensor_tensor(
                out=n1, in0=y_sb, scalar=float(H), in1=musum,
                op0=ALU.mult, op1=ALU.subtract)
            n2 = work.tile([H, G2], FP, tag="n2", name=f"n2{eg}")
            nc.vector.tensor_tensor(out=n2, in0=n1, in1=r_prev,
                                    op=ALU.mult)
            nc.vector.tensor_scalar(outt[:, sl], n2, gam_col, bet_col,
                                    ALU.mult, ALU.add)
            nc.sync.dma_start(out=out[:, sl], in_=outt[:, sl])

    nc.finalize()
    return nc


def _get_program(npad, nr, nc_chunks):
    key = (npad, nr, nc_chunks)
    if _cache.get("key") != key:
        _cache["nc"] = _build_program(npad, nr, nc_chunks)
        _cache["key"] = key
    return _cache["nc"]


def _silu_np(x):
    return x / (1.0 + np.exp(-x))


def kernel(x, adj_dist, mask, cond_vec, W1, b1, W2, b2, W3, b3, W4, b4,
           gamma, beta):
    x = np.asarray(x, dtype=np.float32)
    adj_dist = np.asarray(adj_dist, dtype=np.float32)
    mask_np = np.asarray(mask)
    cond_vec = np.asarray(cond_vec, dtype=np.float32)
    W1 = np.asarray(W1, dtype=np.float32)
    W2 = np.asarray(W2, dtype=np.float32)
    W3 = np.asarray(W3, dtype=np.float32)
    W4 = np.asarray(W4, dtype=np.float32)
    b1 = np.asarray(b1, dtype=np.float32)
    b2 = np.asarray(b2, dtype=np.float32)
    b3 = np.asarray(b3, dtype=np.float32)
    b4 = np.asarray(b4, dtype=np.float32)
    gamma = np.asarray(gamma, dtype=np.float32)
    beta = np.asarray(beta, dtype=np.float32)

    def cb16(a):
        return np.ascontiguousarray(np.asarray(a).astype(ml_bf16))

    jidx = [np.nonzero(mask_np[b])[0] for b in range(B)]
    lmax = max(1, max(len(j) for j in jidx))
    npad = ((lmax + 7) // 8) * 8
    nr = max(1, 512 // npad)
    nc_chunks = (NI + nr - 1) // nr
    KB = 32 + nr
    W = nr * npad
    CW = W + H
    nfull = NI // nr
    rem = NI - nfull * nr

    W1a = W1[:, 0:H]
    W1b = W1[:, H:2 * H]
    W1d = W1[:, 2 * H:2 * H + R]
    W1c = W1[:, 2 * H + R:]
    W3a = W3[:, 0:H]
    W3b = W3[:, H:2 * H]
    sign = 1.0 if (NEWT % 2 == 0) else -1.0
    gam_eff = gamma * (sign / np.sqrt(float(H)))

    onehot = np.zeros((nr, W), dtype=np.float32)
    for e in range(nr):
        onehot[e, e * npad:(e + 1) * npad] = 1.0

    slab_chunks = [min(2, nc_chunks)]
    while sum(slab_chunks) < nc_chunks:
        slab_chunks.append(min(4, nc_chunks - sum(slab_chunks)))

    in_maps = []
    for core in range(8):
        b, ih = core // 2, core % 2
        i0 = ih * NI
        ji = jidx[b]
        L = len(ji)

        xi = x[b, i0:i0 + NI]                      # [NI, H]
        xiT = xi.T                                 # [H, NI]
        xTm = np.zeros((H, npad), dtype=np.float32)
        xTm[:, 0:L] = x[b, ji].T

        # host-folded constants
        trow = W1c @ cond_vec[b] + b1              # [H]
        ACb = W1a @ xiT + trow[:, None]            # [H, NI]
        korr = (npad - L) * _silu_np(ACb)          # [H, NI]
        negd = -(W2 @ korr) + L * b2[:, None]      # [H, NI]
        e_c = W3a @ xiT + b3[:, None]              # [H, NI]
        xbT = xiT + b4[:, None]                    # [H, NI]
        ACbT = ACb.T                               # [NI, H]

        bb1_ = np.concatenate([W1b.T, xTm], axis=1)
        bb2_ = np.concatenate([np.eye(H, dtype=np.float32), W2.T, W3b.T,
                               W4.T, negd, e_c, xbT], axis=1)
        cb_ = np.stack([gam_eff, beta], axis=1)    # [H, 2]

        # per-chunk [KB, CW] = [rhs (adjT + onehot) | lhsT (w1dT + ACbT)]
        adjc = np.zeros((NI, npad, R), dtype=np.float32)
        adjc[:, 0:L, :] = adj_dist[b, i0:i0 + NI][:, ji, :]
        chunks = np.zeros((nc_chunks, KB, CW), dtype=np.float32)
        for cc in range(nc_chunks):
            g0 = cc * nr
            ng = min(nr, NI - g0)
            blk = adjc[g0:g0 + ng]                 # [ng, npad, R]
            chunks[cc, 0:32, 0:ng * npad] = (
                blk.transpose(2, 0, 1).reshape(R, ng * npad))
            chunks[cc, 32:32 + ng, 0:W] = onehot[0:ng]
            chunks[cc, 0:32, W:W + H] = W1d.T
            chunks[cc, 32:32 + ng, W:W + H] = ACbT[g0:g0 + ng]

        m = dict(bb1=cb16(bb1_), bb2=cb16(bb2_),
                 cb=np.ascontiguousarray(cb_, dtype=np.float32))
        c0 = 0
        for s, cnt in enumerate(slab_chunks):
            sl = chunks[c0:c0 + cnt]               # [cnt, KB, CW]
            m[f"slab{s}"] = cb16(
                sl.transpose(1, 0, 2).reshape(KB, cnt * CW))
            c0 += cnt
        in_maps.append(m)

    nc = _get_program(npad, nr, nc_chunks)
    _cache["in_maps"] = in_maps
    res = run_bass_kernel_spmd(nc, in_maps, list(range(8)))

    out_full = np.empty((B, N, H), dtype=np.float32)
    for core in range(8):
        b, ih = core // 2, core % 2
        out_full[b, ih * NI:(ih + 1) * NI] = res.results[core]["out"].T
    return out_full
